# revision 7
# baseline (speedup 1.0000x reference)
"""Grouped-Query Attention (B=2, S=2048, d_model=2048, 32 heads x 64, 8 KV groups)
on 8 Trainium2 NeuronCores.

Sharding: 2D mesh (batch=2) x (tensor-parallel=4 over heads).
Core c = 4*b + tp handles batch b, heads [8*tp, 8*tp+8), KV groups [2*tp, 2*tp+2).
Each core computes a PARTIAL output (its heads' contribution through W_o),
transposed as (d_model, seq); the host sums the 4 TP partials per batch and
transposes back.

Device kernel layout (per core), everything f32 with f32r matmuls:
  inputs (host pre-transposed so d_model lands on SBUF partitions):
    xT   (2048, 2048)  = x[b].T
    wqT  (2048, 512)   = W_q[tp*512:(tp+1)*512].T
    wkT  (2048, 128)   = W_k[tp*128:(tp+1)*128].T
    wvT  (2048, 128)   = W_v[tp*128:(tp+1)*128].T
    woT  (512, 2048)   = W_o[:, tp*512:(tp+1)*512].T
  output:
    outT (2048, 2048)  = partial (context_local @ W_o_local.T).T

  Pipeline: K^T/V^T projections -> V transposed via PE into V_aug tiles
  ([V | ones] so the context matmul also produces softmax sums) -> per
  q-block: Q^T projection, scores^T = K @ Q^T (two heads packed into the
  128-row PE array at partition offsets 0/64), exp on ACT (scale=1/8 fused),
  context^T = V_aug.T @ P^T accumulated over kseq chunks, normalization by
  the ridden-along sums, then the W_o partial projection.
"""

import numpy as np

# Problem constants (hardcoded; kernel.py must be self-contained).
D = 2048          # d_model
S = 2048          # sequence length
B = 2             # batch
DH = 64           # head dim
TP = 4            # tensor-parallel cores per batch
N_CORES = 8
QL = 512          # local q dims (8 heads x 64)
KL = 128          # local kv dims (2 groups x 64)
GL = 2            # local kv groups
NCH = D // 128    # 16 contraction chunks
NKT = S // 128    # 16 key-seq tiles
NQB = S // 512    # 4 q blocks

_NC = None


def _build_nc():
    import concourse.mybir as mybir
    import concourse.tile as tile
    from concourse import bacc
    from concourse.masks import make_identity
    from contextlib import ExitStack

    f32 = mybir.dt.float32
    f32r = mybir.dt.float32r
    EXP = mybir.ActivationFunctionType.Exp

    nc = bacc.Bacc()
    xT = nc.dram_tensor("xT", [D, S], f32r, kind="ExternalInput")
    wqT = nc.dram_tensor("wqT", [D, QL], f32r, kind="ExternalInput")
    wkT = nc.dram_tensor("wkT", [D, KL], f32r, kind="ExternalInput")
    wvT = nc.dram_tensor("wvT", [D, KL], f32r, kind="ExternalInput")
    woT = nc.dram_tensor("woT", [QL, D], f32r, kind="ExternalInput")
    outT = nc.dram_tensor("outT", [D, S], f32, kind="ExternalOutput")

    with tile.TileContext(nc) as tc, ExitStack() as ctx:
        persist = ctx.enter_context(tc.tile_pool(name="persist", bufs=1))
        QT = [persist.tile([128, S], f32r, tag=f"qt{i}", name=f"qt{i}") for i in range(4)]
        Kdup = [persist.tile([128, S], f32r, tag=f"kdup{g}", name=f"kdup{g}") for g in range(GL)]
        Vaug = [[persist.tile([128, DH + 1], f32r, tag=f"vaug{g}_{t}", name=f"vaug{g}_{t}")
                 for t in range(NKT)] for g in range(GL)]
        CTX = [persist.tile([128, S], f32r, tag=f"ctx{i}", name=f"ctx{i}") for i in range(4)]
        ident = persist.tile([128, 128], f32, tag="ident")
        ones = persist.tile([1, DH], f32r, tag="ones")
        onecol_f = persist.tile([128, 1], f32, tag="onecol_f")
        make_identity(nc, ident[:, :])
        nc.gpsimd.memset(onecol_f[:, :], 1.0)
        nc.vector.tensor_copy(ones[:, :], onecol_f[0:1, 0:1].broadcast_to((1, DH)))
        for g in range(GL):
            for t in range(NKT):
                nc.vector.tensor_copy(Vaug[g][t][:, DH:DH + 1], onecol_f[:, :])

        # PSUM: mm512 2 banks, sa/sb 2 banks each, ca/cb 1 bank each = 8 banks
        psmm = ctx.enter_context(tc.tile_pool(name="psmm", bufs=2, space="PSUM"))
        pssc = ctx.enter_context(tc.tile_pool(name="pssc", bufs=1, space="PSUM"))
        psctx = ctx.enter_context(tc.tile_pool(name="psctx", bufs=1, space="PSUM"))

        # ---------- Phase 1: K^T / V^T projections, V transpose, Kdup ----------
        with tc.tile_pool(name="wkvp", bufs=1) as wkvp, \
             tc.tile_pool(name="xt1", bufs=18) as xt1:
            wk = [wkvp.tile([128, KL], f32r, tag=f"wk{k}", name=f"wk{k}") for k in range(NCH)]
            wv = [wkvp.tile([128, KL], f32r, tag=f"wv{k}", name=f"wv{k}") for k in range(NCH)]
            KT = wkvp.tile([128, S], f32r, tag="ktb")
            VT = wkvp.tile([128, S], f32, tag="vtb")
            for k in range(NCH):
                nc.sync.dma_start(wk[k][:, :], wkT[128 * k:128 * (k + 1), :])
                nc.sync.dma_start(wv[k][:, :], wvT[128 * k:128 * (k + 1), :])
            for sb_i in range(NQB):
                ssl = slice(512 * sb_i, 512 * (sb_i + 1))
                xts = [xt1.tile([128, 512], f32r, tag="xt", name="xt") for _ in range(NCH)]
                for k in range(NCH):
                    nc.sync.dma_start(xts[k][:, :], xT[128 * k:128 * (k + 1), ssl])
                kps = psmm.tile([128, 512], f32, tag="mm512")
                for k in range(NCH):
                    nc.tensor.matmul(kps[:, :], wk[k][:, :],
                                     xts[k][:, :],
                                     start=(k == 0), stop=(k == NCH - 1))
                nc.vector.tensor_copy(KT[:, ssl], kps[:, :])
                vps = psmm.tile([128, 512], f32, tag="mm512")
                for k in range(NCH):
                    nc.tensor.matmul(vps[:, :], wv[k][:, :],
                                     xts[k][:, :],
                                     start=(k == 0), stop=(k == NCH - 1))
                nc.vector.tensor_copy(VT[:, ssl], vps[:, :])

            # V natural chunks via PE transpose; build V_aug; duplicate K rows.
            for t in range(NKT):
                trp = psmm.tile([128, 128], f32, tag="mm512")
                nc.tensor.transpose(trp[:, :], VT[:, 128 * t:128 * (t + 1)],
                                    ident[:, :])
                for g in range(GL):
                    nc.vector.tensor_copy(Vaug[g][t][:, 0:DH],
                                          trp[:, 64 * g:64 * (g + 1)])
            for g in range(GL):
                nc.vector.tensor_copy(Kdup[g][0:64, :], KT[64 * g:64 * (g + 1), :])
                nc.vector.tensor_copy(Kdup[g][64:128, :], KT[64 * g:64 * (g + 1), :])

        # ---------- Phase 2: per q-block Q-proj, attention, out-proj ----------
        with tc.tile_pool(name="wqp", bufs=1) as wqp, \
             tc.tile_pool(name="wop", bufs=1) as wop, \
             tc.tile_pool(name="xt2", bufs=8) as xt2, \
             tc.tile_pool(name="ptp", bufs=3) as ptp, \
             tc.tile_pool(name="recp", bufs=4) as recp, \
             tc.tile_pool(name="outp", bufs=3) as outp:
            wq = [wqp.tile([128, QL], f32r, tag=f"wq{k}", name=f"wq{k}") for k in range(NCH)]
            for k in range(NCH):
                nc.sync.dma_start(wq[k][:, :], wqT[128 * k:128 * (k + 1), :])
            wo = [wop.tile([128, S], f32r, tag=f"wo{c}", name=f"wo{c}") for c in range(4)]
            for c in range(4):
                nc.sync.dma_start(wo[c][:, :], woT[128 * c:128 * (c + 1), :])

            for qb in range(NQB):
                qsl = slice(512 * qb, 512 * (qb + 1))

                # Q^T projection for this q block: 2 qtiles per pass x 2 passes
                for pas in range(2):
                    xts = [xt2.tile([128, 512], f32r, tag="xt2", name="xt2")
                           for _ in range(NCH)]
                    for k in range(NCH):
                        nc.sync.dma_start(xts[k][:, :],
                                          xT[128 * k:128 * (k + 1), qsl])
                    qpss = [psmm.tile([128, 512], f32, tag="mm512", name="qps")
                            for _ in range(2)]
                    for k in range(NCH):
                        for j in range(2):
                            qt = 2 * pas + j
                            nc.tensor.matmul(
                                qpss[j][:, :],
                                wq[k][:, 128 * qt:128 * (qt + 1)],
                                xts[k][:, :],
                                start=(k == 0), stop=(k == NCH - 1))
                    for j in range(2):
                        qt = 2 * pas + j
                        nc.vector.tensor_copy(QT[qt][:, qsl], qpss[j][:, :])

                # Attention per head pair p (heads 2p, 2p+1; local group p//2)
                for p in range(4):
                    g = p // 2
                    ca = psctx.tile([DH + 1, 512], f32, tag="ca")
                    cb = psctx.tile([DH + 1, 512], f32, tag="cb")
                    for ktg in range(NKT // 2):
                        sa = pssc.tile([128, 1024], f32, tag="sa")
                        sb_ = pssc.tile([128, 1024], f32, tag="sb")
                        for j in range(2):
                            kt = 2 * ktg + j
                            ksl = slice(128 * kt, 128 * (kt + 1))
                            jsl = slice(512 * j, 512 * (j + 1))
                            nc.tensor.matmul(
                                sa[:, jsl],
                                Kdup[g][0:64, ksl],
                                QT[p][0:64, qsl],
                                start=True, stop=True)
                            nc.tensor.matmul(
                                sb_[:, jsl],
                                Kdup[g][64:128, ksl],
                                QT[p][64:128, qsl],
                                start=True, stop=True)
                        pta = ptp.tile([128, 1024], f32r, tag="pt")
                        ptb = ptp.tile([128, 1024], f32r, tag="pt")
                        nc.scalar.activation(pta[:, :], sa[:, :], EXP, scale=0.125)
                        nc.scalar.activation(ptb[:, :], sb_[:, :], EXP, scale=0.125)
                        for j in range(2):
                            kt = 2 * ktg + j
                            jsl = slice(512 * j, 512 * (j + 1))
                            nc.tensor.matmul(
                                ca[:, :], Vaug[g][kt][:, :],
                                pta[:, jsl],
                                start=(kt == 0), stop=(kt == NKT - 1))
                            nc.tensor.matmul(
                                cb[:, :], Vaug[g][kt][:, :],
                                ptb[:, jsl],
                                start=(kt == 0), stop=(kt == NKT - 1))
                    # normalize by sums (row DH of ca/cb) into CTX
                    for h_i, cps in ((0, ca), (1, cb)):
                        rec = recp.tile([1, 512], f32r, tag="rec")
                        with nc.allow_low_precision(reason="softmax 1/sum in f32r"):
                            nc.vector.reciprocal(rec[:, :], cps[DH:DH + 1, :])
                        bc = psmm.tile([64, 512], f32, tag="mm512")
                        nc.tensor.matmul(bc[:, :], ones[:, :],
                                         rec[:, :],
                                         start=True, stop=True)
                        bcs = recp.tile([64, 512], f32, tag="bcs", name="bcs")
                        nc.vector.tensor_copy(bcs[:, :], bc[:, :])
                        nc.vector.tensor_mul(
                            CTX[p][64 * h_i:64 * (h_i + 1), qsl],
                            cps[0:DH, :], bcs[:, :])

                # Partial W_o projection for this q block
                for ot in range(NCH):
                    ops_ = psmm.tile([128, 512], f32, tag="mm512")
                    for c in range(4):
                        nc.tensor.matmul(
                            ops_[:, :],
                            wo[c][:, 128 * ot:128 * (ot + 1)],
                            CTX[c][:, qsl],
                            start=(c == 0), stop=(c == 3))
                    osb = outp.tile([128, 512], f32, tag="ob")
                    nc.vector.tensor_copy(osb[:, :], ops_[:, :])
                    nc.sync.dma_start(outT[128 * ot:128 * (ot + 1), qsl],
                                      osb[:, :])

    nc.compile()
    return nc


def _get_nc():
    global _NC
    if _NC is None:
        _NC = _build_nc()
    return _NC


def _shard_inputs(x, W_q, W_k, W_v, W_o):
    x = np.asarray(x, dtype=np.float32)
    W_q = np.asarray(W_q, dtype=np.float32)
    W_k = np.asarray(W_k, dtype=np.float32)
    W_v = np.asarray(W_v, dtype=np.float32)
    W_o = np.asarray(W_o, dtype=np.float32)
    in_maps = []
    for c in range(N_CORES):
        b, tp = divmod(c, TP)
        in_maps.append({
            "xT": np.ascontiguousarray(x[b].T),
            "wqT": np.ascontiguousarray(W_q[QL * tp:QL * (tp + 1), :].T),
            "wkT": np.ascontiguousarray(W_k[KL * tp:KL * (tp + 1), :].T),
            "wvT": np.ascontiguousarray(W_v[KL * tp:KL * (tp + 1), :].T),
            "woT": np.ascontiguousarray(W_o[:, QL * tp:QL * (tp + 1)].T),
        })
    return in_maps


def kernel(x, W_q, W_k, W_v, W_o):
    from concourse.bass_utils import run_bass_kernel_spmd

    nc = _get_nc()
    in_maps = _shard_inputs(x, W_q, W_k, W_v, W_o)
    res = run_bass_kernel_spmd(nc, in_maps, list(range(N_CORES)))
    out = np.empty((B, S, D), dtype=np.float32)
    for b in range(B):
        acc = res.results[TP * b]["outT"].copy()
        for tp in range(1, TP):
            acc += res.results[TP * b + tp]["outT"]
        out[b] = acc.T
    return out
